# revision 13
# baseline (speedup 1.0000x reference)
"""GAT (2-layer, 4-head) Trainium2 kernel, 8-core SPMD.

Strategy:
  - Nodes partitioned across 8 cores by destination (6250 each).
  - Tables are feat-only (x@W) 256B f16 rows stored in degree-sorted
    (permuted) order shared by BOTH layers; attention logits el/er are
    computed on-device from the gathered/local features (al/ar dots),
    so gather rows shrink from 512B to 256B and the slot structure
    (perm/idx/maskbias) is built once and reused for both layers.
  - Per layer: each core computes the feature table for its node slice,
    AllGather -> full table in every core's DRAM (Shared addr space).
  - Edges laid out dst-major: each aggregation tile assigns one dst node
    per SBUF partition (degree-balanced permutation); dma_gather pulls
    table[src] rows into slots; softmax numerator+denominator reduced
    over slot columns via f16 identity-matmul PSUM accumulation (ex
    appended as 4 extra columns so den rides the same accumulation).
  - int16 gather indices: slots split into lo (<32768) and hi column
    blocks gathered from offset table views.
"""

import sys

sys.path.insert(0, "/opt/trn_rl_repo")

import numpy as np

N_CORES = 8
N_NODES = 50000
NPC = N_NODES // N_CORES  # 6250
IN_DIM = 128
HEADS = 4
DIM = 32
HD = HEADS * DIM  # 128
EW = 128          # fp16 elements per table row (256B)
HALF = 32768      # int16 gather index limit
P = 128
TILES = (NPC + P - 1) // P  # 49
GROUP_COLS = 64
CALL_COLS = 7      # rows per dma_gather call = CALL_COLS*128
NEG_BIG = -1.0e30
EPS = 1e-30


# ----------------------------------------------------------------------------
# host-side slot building (index metadata only)
# ----------------------------------------------------------------------------

def _wrap_idx(idx_flat):
    """[n] -> [128, n/16] int16: i at [i%16 (replicated x8), i//16]."""
    n = idx_flat.shape[0]
    assert n % 16 == 0
    w = idx_flat.reshape(n // 16, 16).T.astype(np.int16)
    return np.tile(w, (8, 1))


def _build_shared(cores_src, cores_dstl):
    """Build the slot structure shared by both layers.

    cores_src: per-core GLOBAL src node ids of that core's edges.
    cores_dstl: per-core local dst ids.
    Tables are stored in permuted order, so the table row of node (k, n)
    is k*NPC + invperm_k[n]; the permutation is chosen by degree sorting
    with lo/hi membership estimated from natural-order rows, then the
    final row ids (and lo/hi split) are derived from the fixed perm.
    Returns (shared, per_core, perms).
    """
    # pass 1: perm from natural-row lo/hi degrees
    perms = []
    for k in range(N_CORES):
        s, d = cores_src[k], cores_dstl[k]
        lo_nat = s < HALF
        lo_deg = np.bincount(d[lo_nat], minlength=NPC)
        hi_deg = np.bincount(d[~lo_nat], minlength=NPC)
        perms.append(np.lexsort((hi_deg, lo_deg)))
    invperm = []
    for k in range(N_CORES):
        ip = np.empty(NPC, dtype=np.int64)
        ip[perms[k]] = np.arange(NPC)
        invperm.append(ip)

    # pass 2: final rows + lo/hi lists under the fixed perms
    percore = []
    for k in range(N_CORES):
        s, d = cores_src[k], cores_dstl[k]
        s_core = s // NPC
        s_loc = s % NPC
        rows = np.empty_like(s)
        for j in range(N_CORES):
            m = s_core == j
            rows[m] = j * NPC + invperm[j][s_loc[m]]
        lo_m = rows < HALF
        lo_lists = [[] for _ in range(NPC)]
        hi_lists = [[] for _ in range(NPC)]
        for r, dd, m in zip(rows, d, lo_m):
            (lo_lists if m else hi_lists)[dd].append(r if m else r - HALF)
        percore.append((lo_lists, hi_lists))

    # per-tile max column counts across cores
    CA = np.zeros(TILES, dtype=np.int64)
    CB = np.zeros(TILES, dtype=np.int64)
    for k in range(N_CORES):
        lo_l, hi_l = percore[k]
        perm = perms[k]
        for t in range(TILES):
            nodes = perm[t * P: min((t + 1) * P, NPC)]
            CA[t] = max(CA[t], max((len(lo_l[n]) for n in nodes), default=0))
            CB[t] = max(CB[t], max((len(hi_l[n]) for n in nodes), default=0))
    CA = np.maximum(CA, 1)
    CB = np.maximum(CB, 1)

    # group tiles under a column budget so the G pool stays bounded
    groups = []
    cur = []
    cur_cols = 0
    for t in range(TILES):
        c = int(CA[t] + CB[t])
        if cur and cur_cols + c > GROUP_COLS:
            groups.append(cur)
            cur = []
            cur_cols = 0
        cur.append(t)
        cur_cols += c
    if cur:
        groups.append(cur)

    out = []
    for k in range(N_CORES):
        lo_l, hi_l = percore[k]
        perm = perms[k]
        idx_blocks = []
        mb_cols = []
        for g in groups:
            LO = int(CA[g].sum())
            HI = int(CB[g].sum())
            ilo = np.zeros(LO * P, dtype=np.int64)
            ihi = np.zeros(HI * P, dtype=np.int64)
            c_lo = 0
            c_hi = 0
            for gi, t in enumerate(g):
                mb_t = np.full((P, int(CA[t] + CB[t])), NEG_BIG, dtype=np.float32)
                for p in range(P):
                    ni = t * P + p
                    if ni >= NPC:
                        continue
                    n = perm[ni]
                    for c, s in enumerate(lo_l[n]):
                        ilo[(c_lo + c) * P + p] = s
                        mb_t[p, c] = 0.0
                    for c, s in enumerate(hi_l[n]):
                        ihi[(c_hi + c) * P + p] = s
                        mb_t[p, CA[t] + c] = 0.0
                c_lo += int(CA[t])
                c_hi += int(CB[t])
                mb_cols.append(mb_t)
            for arr in (ilo, ihi):
                cols = arr.shape[0] // P
                for c0 in range(0, cols, CALL_COLS):
                    c1 = min(c0 + CALL_COLS, cols)
                    idx_blocks.append(_wrap_idx(arr[c0 * P:c1 * P]))
        idx = np.concatenate(idx_blocks, axis=1)
        mb = np.concatenate(mb_cols, axis=1)
        out.append({"idx": idx, "mb": mb, "perm": perm})

    shared = {"CA": CA, "CB": CB, "groups": groups}
    return shared, out


# ----------------------------------------------------------------------------
# device program
# ----------------------------------------------------------------------------

def _build_program(sh, IC, CE):
    import concourse.bass as bass
    import concourse.bacc as bacc
    import concourse.tile as tile
    from concourse import mybir, library_config
    from concourse.masks import make_identity

    f32 = mybir.dt.float32
    f16 = mybir.dt.float16
    i16 = mybir.dt.int16
    Alu = mybir.AluOpType
    Act = mybir.ActivationFunctionType
    HD4 = HD + 4

    nc = bacc.Bacc("TRN2", target_bir_lowering=False, debug=False,
                   enable_asserts=True, num_devices=N_CORES, num_swdge_queues=4,
                   dynamic_dma_scratch_size=32768)

    xts = nc.dram_tensor("xts", [P, NPC], f32, kind="ExternalInput")
    W1 = nc.dram_tensor("W1", [IN_DIM, HD], f32, kind="ExternalInput")
    W2 = nc.dram_tensor("W2", [HD, HD], f32, kind="ExternalInput")
    alr1 = nc.dram_tensor("alr1", [P, HD], f16, kind="ExternalInput")
    arr1 = nc.dram_tensor("arr1", [P, HD], f16, kind="ExternalInput")
    alr2 = nc.dram_tensor("alr2", [P, HD], f16, kind="ExternalInput")
    arr2 = nc.dram_tensor("arr2", [P, HD], f16, kind="ExternalInput")
    b1r = nc.dram_tensor("b1r", [P, HD], f32, kind="ExternalInput")
    b2r = nc.dram_tensor("b2r", [P, HD], f32, kind="ExternalInput")
    idx = nc.dram_tensor("idx", [P, IC], i16, kind="ExternalInput")
    mbt = nc.dram_tensor("mbt", [P, CE], f32, kind="ExternalInput")
    out_d = nc.dram_tensor("out", [NPC, DIM], f32, kind="ExternalOutput")

    with tile.TileContext(nc) as tc:
        with (
            tc.tile_pool(name="const", bufs=1) as cpool,
            tc.tile_pool(name="sb", bufs=2) as sb,
            tc.tile_pool(name="gpool", bufs=2) as gpool,
            tc.tile_pool(name="mpool", bufs=2) as mpool,
            tc.tile_pool(name="stat", bufs=1) as stat,
            tc.tile_pool(name="ps", bufs=2, space="PSUM") as ps,
            tc.tile_pool(name="pst", bufs=2, space="PSUM") as pst,
            tc.tile_pool(name="dram", bufs=1, space="DRAM") as dram,
        ):
            nc.gpsimd.load_library(library_config.mlp)

            ident = cpool.tile([P, P], f32)
            make_identity(nc, ident[:])
            ident16 = cpool.tile([P, P], f16)
            make_identity(nc, ident16[:])

            # ---- shared constants
            b1_sb = cpool.tile([P, HD], f32)
            nc.sync.dma_start(b1_sb[:], b1r[:])
            b2_sb = cpool.tile([P, HD], f32)
            nc.sync.dma_start(b2_sb[:], b2r[:])
            b2mean = cpool.tile([P, DIM], f32)
            nc.vector.tensor_reduce(
                out=b2mean[:], in_=b2_sb[:].rearrange("p (h j) -> p j h", h=HEADS),
                op=Alu.add, axis=mybir.AxisListType.X)
            nc.vector.tensor_scalar_mul(b2mean[:], b2mean[:], 0.25)

            W1_sb = cpool.tile([P, HD], f32)
            nc.sync.dma_start(W1_sb[:], W1[:])
            W2_sb = cpool.tile([P, HD], f32)
            nc.sync.dma_start(W2_sb[:], W2[:])
            al1_sb = cpool.tile([P, HD], f16)
            nc.sync.dma_start(al1_sb[:], alr1[:])
            ar1_sb = cpool.tile([P, HD], f16)
            nc.sync.dma_start(ar1_sb[:], arr1[:])
            al2_sb = cpool.tile([P, HD], f16)
            nc.sync.dma_start(al2_sb[:], alr2[:])
            ar2_sb = cpool.tile([P, HD], f16)
            nc.sync.dma_start(ar2_sb[:], arr2[:])

            # ---- DRAM tables
            t1_slice = dram.tile([NPC, EW], f16)
            t1_full = dram.tile([N_NODES, EW], f16, addr_space="Shared")
            t2_slice = dram.tile([NPC, EW], f16)
            t2_full = dram.tile([N_NODES, EW], f16, addr_space="Shared")

            h_tiles = stat.tile([P, TILES * HD], f32)
            out_sb = stat.tile([P, TILES * DIM], f32)
            fsl1 = stat.tile([P, TILES * HD], f16)
            fsl2 = stat.tile([P, TILES * HD], f16)
            # zero the ragged last-tile region (er reads it; avoid NaNs)
            nc.gpsimd.memset(fsl1[:, (TILES - 1) * HD:], 0.0)
            nc.gpsimd.memset(fsl2[:, (TILES - 1) * HD:], 0.0)

            idx_sb = stat.tile([P, IC], i16)
            nc.sync.dma_start(idx_sb[:], idx[:])
            mb_sb = stat.tile([P, CE], f32)
            nc.sync.dma_start(mb_sb[:], mbt[:])

            # ---- table phase: feat = x @ W, f16 rows + SBUF copy for er
            def table_tile(t, lhs_cols, W_t, tslice, fsl):
                n0 = t * P
                w = min(n0 + P, NPC) - n0
                tps = pst.tile([P, HD], f32, space="PSUM", tag="tbps")
                nc.tensor.matmul(out=tps[:w, :], lhsT=lhs_cols[:, :w], rhs=W_t[:],
                                 start=True, stop=True)
                tb = sb.tile([P, EW], f16, tag="tb")
                nc.scalar.copy(tb[:w, :], tps[:w, :])
                nc.vector.tensor_copy(fsl[:w, t * HD:(t + 1) * HD], tps[:w, :])
                nc.sync.dma_start(tslice[n0:n0 + w, :], tb[:w, :])

            for t in range(TILES):
                n0 = t * P
                w = min(n0 + P, NPC) - n0
                xt_sb = sb.tile([P, P], f32, tag="xt")
                nc.sync.dma_start(xt_sb[:, :w], xts[:, n0:n0 + w])
                table_tile(t, xt_sb[:, :w], W1_sb, t1_slice, fsl1)

            nc.gpsimd.collective_compute(
                "AllGather", Alu.bypass,
                replica_groups=[list(range(N_CORES))],
                ins=[t1_slice[:]], outs=[t1_full[:]])

            # ---- aggregation phase (shared for both layers)
            def agg_layer(tfull, fsl, al_sb, ar_sb, epilogue):
                CA, CB, groups = sh["CA"], sh["CB"], sh["groups"]
                io = 0   # idx column offset (int16 cols)
                eo = 0   # maskbias / e-col offset
                ti = 0   # global tile index
                q = 0
                for g in groups:
                    LO = int(CA[g].sum())
                    HI = int(CB[g].sum())
                    ncols = LO + HI
                    G = gpool.tile([P, ncols, EW], f16, tag="G")
                    for blk, view, cnt in (
                        (0, tfull[:], LO),
                        (LO, tfull[HALF:, :], HI),
                    ):
                        for c0 in range(0, cnt, CALL_COLS):
                            c1 = min(c0 + CALL_COLS, cnt)
                            n = (c1 - c0) * P
                            nc.gpsimd.dma_gather(
                                G[:, blk + c0:blk + c1, :], view,
                                idx_sb[:, io:io + n // 16], n, n, EW,
                                queue_num=q % 4)
                            io += n // 16
                            q += 1
                    lo0 = 0
                    hi0 = LO
                    for gi, t in enumerate(g):
                        ca, cb = int(CA[t]), int(CB[t])
                        cc = ca + cb
                        # --- er from local slice feats
                        ert = sb.tile([P, HD], f16, tag="ert")
                        nc.vector.tensor_tensor(
                            out=ert[:], in0=fsl[:, t * HD:(t + 1) * HD],
                            in1=ar_sb[:], op=Alu.mult)
                        er_t = sb.tile([P, HEADS], f32, tag="er")
                        nc.vector.tensor_reduce(
                            out=er_t[:],
                            in_=ert[:].rearrange("p (h j) -> p h j", j=DIM),
                            op=Alu.add, axis=mybir.AxisListType.X)
                        # --- el from gathered feats (M reused as scratch)
                        M = mpool.tile([P, cc * HD4], f16, tag="M")
                        M4 = M[:].rearrange("p (c x) -> p c x", x=HD4)
                        for (o0, n0, c0) in ((0, ca, lo0), (ca, cb, hi0)):
                            nc.vector.tensor_tensor(
                                out=M4[:, o0:o0 + n0, 0:HD],
                                in0=G[:, c0:c0 + n0, :],
                                in1=al_sb[:].unsqueeze(1)
                                    .to_broadcast([P, n0, HD]),
                                op=Alu.mult)
                        el_t = sb.tile([P, cc * HEADS], f32, tag="el")
                        nc.vector.tensor_reduce(
                            out=el_t[:],
                            in_=M4[:, :, 0:HD]
                                .rearrange("p c (h j) -> p (c h) j", j=DIM),
                            op=Alu.add, axis=mybir.AxisListType.X)
                        # --- attention logits
                        e_t = sb.tile([P, cc * HEADS], f32, tag="e")
                        e3 = e_t[:].rearrange("p (c h) -> p c h", h=HEADS)
                        nc.vector.tensor_tensor(
                            out=e3[:],
                            in0=el_t[:].rearrange("p (c h) -> p c h", h=HEADS),
                            in1=er_t[:].unsqueeze(1)
                                .to_broadcast([P, cc, HEADS]),
                            op=Alu.add)
                        # leaky_relu + mask bias
                        nc.vector.scalar_tensor_tensor(
                            out=e_t[:], in0=e_t[:], scalar=0.2, in1=e_t[:],
                            op0=Alu.mult, op1=Alu.max)
                        nc.vector.tensor_tensor(
                            out=e3[:],
                            in0=e3[:],
                            in1=mb_sb[:, eo:eo + cc].unsqueeze(2)
                                .to_broadcast([P, cc, HEADS]),
                            op=Alu.add)
                        ex_t = sb.tile([P, cc * HEADS], f16, tag="ex")
                        nc.scalar.activation(ex_t[:], e_t[:], Act.Exp)
                        ex3 = ex_t[:].rearrange("p (c h) -> p c h", h=HEADS)
                        # --- scaled messages, ex appended for fused denom
                        for (o0, n0, c0) in ((0, ca, lo0), (ca, cb, hi0)):
                            nc.vector.tensor_tensor(
                                out=M4[:, o0:o0 + n0, 0:HD]
                                    .rearrange("p c (h j) -> p c h j", j=DIM),
                                in0=G[:, c0:c0 + n0, :]
                                    .rearrange("p c (h j) -> p c h j", j=DIM),
                                in1=ex3[:, o0:o0 + n0].unsqueeze(3)
                                    .to_broadcast([P, n0, HEADS, DIM]),
                                op=Alu.mult)
                        nc.scalar.copy(M4[:, :, HD:HD4], ex3[:])
                        # --- identity-matmul reduce over slot columns
                        num_ps = ps.tile([P, HD4], f32, space="PSUM", tag="num")
                        for c in range(cc):
                            nc.tensor.matmul(
                                out=num_ps[:], lhsT=ident16[:],
                                rhs=M[:, c * HD4:(c + 1) * HD4],
                                start=(c == 0), stop=(c == cc - 1))
                        epilogue(ti, num_ps)
                        lo0 += ca
                        hi0 += cb
                        eo += cc
                        ti += 1

            # ---- layer-1 epilogue: h = elu(num/den + b1)
            def epi1(t, num_ps):
                dent = sb.tile([P, HEADS], f32, tag="dent")
                nc.vector.tensor_scalar_add(dent[:], num_ps[:, HD:HD + 4], EPS)
                rcp = sb.tile([P, HEADS], f32, tag="rcp")
                nc.vector.reciprocal(rcp[:], dent[:])
                h0 = sb.tile([P, HD], f32, tag="h0")
                nc.vector.tensor_tensor(
                    out=h0[:].rearrange("p (h j) -> p h j", j=DIM),
                    in0=num_ps[:, 0:HD].rearrange("p (h j) -> p h j", j=DIM),
                    in1=rcp[:].unsqueeze(2).to_broadcast([P, HEADS, DIM]),
                    op=Alu.mult)
                nc.vector.tensor_tensor(out=h0[:], in0=h0[:], in1=b1_sb[:],
                                        op=Alu.add)
                ext = sb.tile([P, HD], f32, tag="hexp")
                nc.scalar.activation(ext[:], h0[:], Act.Exp)
                u = sb.tile([P, HD], f32, tag="hu")
                nc.vector.tensor_scalar(
                    out=u[:], in0=ext[:], scalar1=1.0, scalar2=0.0,
                    op0=Alu.subtract, op1=Alu.min)
                nc.vector.scalar_tensor_tensor(
                    out=h_tiles[:, t * HD:(t + 1) * HD], in0=h0[:], scalar=0.0,
                    in1=u[:], op0=Alu.max, op1=Alu.add)

            agg_layer(t1_full, fsl1, al1_sb, ar1_sb, epi1)

            # ---- layer-2 table phase (from h tiles)
            for t in range(TILES):
                hT_ps = pst.tile([P, P], f32, space="PSUM", tag="hT")
                nc.tensor.transpose(
                    hT_ps[:], h_tiles[:, t * HD:(t + 1) * HD], ident[:])
                hT_sb = sb.tile([P, P], f32, tag="hTs")
                nc.vector.tensor_copy(hT_sb[:], hT_ps[:])
                table_tile(t, hT_sb[:, :P], W2_sb, t2_slice, fsl2)

            nc.gpsimd.collective_compute(
                "AllGather", Alu.bypass,
                replica_groups=[list(range(N_CORES))],
                ins=[t2_slice[:]], outs=[t2_full[:]])

            # ---- layer-2 epilogue: out = mean_h(num/den) + mean(b2)
            def epi2(t, num_ps):
                dent = sb.tile([P, HEADS], f32, tag="dent")
                nc.vector.tensor_scalar(
                    out=dent[:], in0=num_ps[:, HD:HD + 4], scalar1=4.0,
                    scalar2=EPS, op0=Alu.mult, op1=Alu.add)
                rcp = sb.tile([P, HEADS], f32, tag="rcp")
                nc.vector.reciprocal(rcp[:], dent[:])
                m0 = sb.tile([P, HD], f32, tag="h0")
                nc.vector.tensor_tensor(
                    out=m0[:].rearrange("p (h j) -> p h j", j=DIM),
                    in0=num_ps[:, 0:HD].rearrange("p (h j) -> p h j", j=DIM),
                    in1=rcp[:].unsqueeze(2).to_broadcast([P, HEADS, DIM]),
                    op=Alu.mult)
                red = sb.tile([P, DIM], f32, tag="red")
                nc.vector.tensor_reduce(
                    out=red[:], in_=m0[:].rearrange("p (h j) -> p j h", h=HEADS),
                    op=Alu.add, axis=mybir.AxisListType.X)
                nc.vector.tensor_tensor(
                    out=out_sb[:, t * DIM:(t + 1) * DIM], in0=red[:],
                    in1=b2mean[:], op=Alu.add)

            agg_layer(t2_full, fsl2, al2_sb, ar2_sb, epi2)

            # ---- write output (tile-slot order; host unpermutes)
            for t in range(TILES):
                n0 = t * P
                w = min(n0 + P, NPC) - n0
                nc.sync.dma_start(
                    out_d[n0:n0 + w, :],
                    out_sb[:w, t * DIM:(t + 1) * DIM])

    nc.compile()
    return nc


# ----------------------------------------------------------------------------
# entry point
# ----------------------------------------------------------------------------

_CACHE = {}
_DEBUG = None


def kernel(inputs, src, dst, W1, al1, ar1, b1, W2, al2, ar2, b2):
    from concourse import bass_utils

    x = np.asarray(inputs, dtype=np.float32)
    src = np.asarray(src).astype(np.int64)
    dst = np.asarray(dst).astype(np.int64)
    W1 = np.asarray(W1, dtype=np.float32)
    W2 = np.asarray(W2, dtype=np.float32)
    al1 = np.asarray(al1, dtype=np.float32)
    ar1 = np.asarray(ar1, dtype=np.float32)
    al2 = np.asarray(al2, dtype=np.float32)
    ar2 = np.asarray(ar2, dtype=np.float32)
    b1 = np.asarray(b1, dtype=np.float32)
    b2 = np.asarray(b2, dtype=np.float32)

    # --- per-core edge bucketing by dst
    core_of = dst // NPC
    dst_local = dst % NPC
    srcs = [src[core_of == k] for k in range(N_CORES)]
    dstl = [dst_local[core_of == k] for k in range(N_CORES)]

    sh, pc = _build_shared(srcs, dstl)
    IC = pc[0]["idx"].shape[1]
    CE = pc[0]["mb"].shape[1]

    key = (IC, CE, tuple(sh["CA"]), tuple(sh["CB"]))
    if key not in _CACHE:
        _CACHE.clear()
        _CACHE[key] = _build_program(sh, IC, CE)
    nc = _CACHE[key]

    xT = np.ascontiguousarray(x.T)
    b1_rep = np.tile(b1.reshape(1, HD), (P, 1)).astype(np.float32)
    b2_rep = np.tile(b2.reshape(1, HD), (P, 1)).astype(np.float32)
    al1_rep = np.tile(al1.reshape(1, HD), (P, 1)).astype(np.float16)
    ar1_rep = np.tile(ar1.reshape(1, HD), (P, 1)).astype(np.float16)
    al2_rep = np.tile(al2.reshape(1, HD), (P, 1)).astype(np.float16)
    ar2_rep = np.tile(ar2.reshape(1, HD), (P, 1)).astype(np.float16)

    in_maps = []
    for k in range(N_CORES):
        xk = xT[:, k * NPC:(k + 1) * NPC]
        in_maps.append({
            "xts": np.ascontiguousarray(xk[:, pc[k]["perm"]]),
            "W1": W1, "W2": W2,
            "alr1": al1_rep, "arr1": ar1_rep,
            "alr2": al2_rep, "arr2": ar2_rep,
            "b1r": b1_rep, "b2r": b2_rep,
            "idx": pc[k]["idx"], "mbt": pc[k]["mb"],
        })

    import os as _os2
    _trace = _os2.environ.get("GAT_TRACE") == "1"
    _tkw = {}
    if _trace:
        _tdir = _os2.environ.get("GAT_TRACE_DIR")
        if _tdir:
            _os2.makedirs(_tdir, exist_ok=True)
            _tkw["tmpdir"] = _tdir
        _tkw["trace"] = True
    res = bass_utils.run_bass_kernel_spmd(
        nc, in_maps, core_ids=list(range(N_CORES)), **_tkw)

    global _DEBUG
    _DEBUG = {"res": res, "pc": pc, "sh": sh}
    out = np.empty((N_NODES, DIM), dtype=np.float32)
    for k in range(N_CORES):
        r = np.asarray(res.results[k]["out"])
        out[k * NPC + pc[k]["perm"]] = r
    return out
